# revision 24
# baseline (speedup 1.0000x reference)
"""Trainium2 Bass kernel for nn_ContinuousPool.

Computes, for x:(32,96,128,128) f32 and pool_strength:(1,96,1,1) f32:
    cur = x
    repeat 10: cur = cur + s * (maxpool3x3_same(cur) - cur)
    out = avgpool2x2(cur)            -> (32,96,64,64)

Strategy (v2):
  - Pure data parallel over 8 cores: 384 images/core as 3 chunks of 128
    (one image per SBUF partition).
  - fp16 on device (host casts x): DVE tensor_tensor runs in 2x_1p mode.
  - Rescaled recurrence  u' = u + (s/(1-s)) * maxpool3x3(u)  so each step
    is 4 tensor_max + 1 scale + 1 add; the scale (w = c*v) runs on the
    Activation engine, off the DVE critical path.
  - All DVE ops are half-frame (N=8192): measured per-op cost is
    sub-linear in N on this silicon (4.49us @8192 vs 10.4us @16384).
  - Chunks 0 and 1 run interleaved (double-buffered u/r/v) so ACT work
    and DVE inter-op gaps overlap; chunk 2 follows solo.
  - w = c*v overlays r's interior (disjoint lifetimes within a step).
  - Final avgpool2x2 in fp16; output scaled on ACT and cast fp16->f32 by
    a gpsimd (SWDGE) casting DMA on the way to DRAM.
"""

import sys

if "/opt/trn_rl_repo" not in sys.path:
    sys.path.insert(0, "/opt/trn_rl_repo")

import numpy as np

B, C, H, W = 32, 96, 128, 128
T = 10
N_CORES = 8
B_PER_CORE = B // N_CORES          # 4
IMGS = B_PER_CORE * C              # 384 images per core
CHUNK = 128                        # images (partitions) per chunk
NCHUNK = IMGS // CHUNK             # 3

UOFF = 2                           # u interior col0 offset (even -> 4B aligned)
USTR = 130                         # u row stride (pad col each side)
USZ = UOFF + USTR * H + 2
RSTR = 128
RSZ = RSTR * (H + 2)               # pad row above and below
VSZ = H * W                        # 16384
HB = H // 2                        # rows per half

_CACHE = {}

# schedule knobs (tuned via TimelineSim sweep, verified on HW)
OPTS = {
    "pads_only_memset": True,   # memset only pad cells, not whole tiles
    "op5a": "act_halves",       # act_halves | act_full | dve_ts
    "op13_full": False,         # op1/op3 as full-frame ops
    "op1_pos": "start",         # start | end (emit op1 after prev op5b)
}


def _build(R=None):
    import concourse.bacc as bacc
    import concourse.mybir as mybir
    from concourse import tile

    f16 = mybir.dt.float16
    f32 = mybir.dt.float32

    nc = bacc.Bacc("TRN2", target_bir_lowering=False, debug=False,
                   num_devices=N_CORES)

    x_d = nc.dram_tensor("x", [IMGS, H * W], f16, kind="ExternalInput")
    c_d = nc.dram_tensor("cvec", [IMGS, 1], f32, kind="ExternalInput")
    f_d = nc.dram_tensor("fvec", [IMGS, 1], f32, kind="ExternalInput")
    y_d = nc.dram_tensor("y", [IMGS, (H // 2) * (W // 2)], f32,
                         kind="ExternalOutput")

    with tile.TileContext(nc, num_cores=N_CORES) as tc:
        with tc.tile_pool(name="main", bufs=1) as pool:
            u_t, r_t, v_t = [], [], []
            for i in range(2):  # pair-local allocation: u,r,v of a chunk
                u_t.append(pool.tile([128, USZ], f16, name=f"u{i}",
                                     tag=f"u{i}"))
                r_t.append(pool.tile([128, RSZ], f16, name=f"r{i}",
                                     tag=f"r{i}"))
                v_t.append(pool.tile([128, VSZ], f16, name=f"v{i}",
                                     tag=f"v{i}"))
            cs_t = pool.tile([128, 2 * NCHUNK], f32, tag="cs")

            NEGINF = float("-inf")
            for i in range(2):
                if OPTS["pads_only_memset"]:
                    # u pads: row i's (prev-right, own-left) pad pair sits at
                    # cells [130i, 130i+1] of a base-0 row view; plus the
                    # last row's right pad at the tile end.
                    up = u_t[i][:, 0:USTR * H].rearrange(
                        "p (h w) -> p h w", h=H, w=USTR)[:, :, 0:2]
                    nc.vector.memset(up, NEGINF)
                    nc.vector.memset(u_t[i][:, USTR * H:USTR * H + 2], NEGINF)
                    # r pad rows (-1 and H)
                    nc.vector.memset(r_t[i][:, 0:RSTR], NEGINF)
                    nc.vector.memset(r_t[i][:, RSTR * (H + 1):], NEGINF)
                else:
                    nc.vector.memset(u_t[i][:, :], NEGINF)
                    nc.vector.memset(r_t[i][:, :], NEGINF)

            def u_view(c, h, dx=0):
                t = u_t[c % 2]
                if h is None:
                    base, n = UOFF + dx, H
                else:
                    base, n = UOFF + dx + USTR * HB * h, HB
                return t[:, base:base + USTR * n].rearrange(
                    "p (h w) -> p h w", h=n, w=USTR)[:, :, 0:W]

            def r_view(c, h, dy=0):
                t = r_t[c % 2]
                if h is None:
                    base, n = RSTR * (1 + dy), H
                else:
                    base, n = RSTR * (1 + dy) + RSTR * HB * h, HB
                return t[:, base:base + RSTR * n].rearrange(
                    "p (h w) -> p h w", h=n, w=RSTR)

            def v_view(c, h):
                t = v_t[c % 2]
                if h is None:
                    return t[:, 0:RSTR * H].rearrange(
                        "p (h w) -> p h w", h=H, w=RSTR)
                return t[:, RSTR * HB * h:RSTR * HB * (h + 1)].rearrange(
                    "p (h w) -> p h w", h=HB, w=RSTR)

            def load_chunk(c):
                # quarter-granular DMAs so the first op1 half waits less
                rows = slice(c * CHUNK, (c + 1) * CHUNK)
                x_v = x_d[rows, :].rearrange("p (h w) -> p h w", h=H, w=W)
                qn = H // 4
                t = u_t[c % 2]
                for q in range(4):
                    dst = t[:, UOFF + USTR * qn * q:
                            UOFF + USTR * qn * (q + 1)].rearrange(
                        "p (h w) -> p h w", h=qn, w=USTR)[:, :, 0:W]
                    nc.sync.dma_start(dst, x_v[:, qn * q:qn * (q + 1), :])

            def emit_op1(c):
                # r = max(u<<1, u>>1) (same-tile reads)
                if OPTS["op13_full"]:
                    nc.vector.tensor_max(r_view(c, None), u_view(c, None, -1),
                                         u_view(c, None, +1))
                else:
                    for h in (0, 1):
                        nc.vector.tensor_max(r_view(c, h), u_view(c, h, -1),
                                             u_view(c, h, +1))

            def emit_steps(chunks, t, last):
                solo = len(chunks) == 1

                def csc(c):
                    return cs_t[:, 2 * (c % NCHUNK):2 * (c % NCHUNK) + 1]

                mode = OPTS["op5a"]
                if OPTS["op1_pos"] == "start":
                    for c in chunks:
                        emit_op1(c)
                for c in chunks:           # op2: r = max(r, u)
                    for h in (0, 1):
                        nc.vector.tensor_max(r_view(c, h), r_view(c, h),
                                             u_view(c, h))
                for c in chunks:           # op3: v = max(r<<W, r>>W)
                    if OPTS["op13_full"]:
                        nc.vector.tensor_max(
                            v_view(c, None), r_view(c, None, -1),
                            r_view(c, None, +1))
                    else:
                        for h in (0, 1):
                            nc.vector.tensor_max(
                                v_view(c, h), r_view(c, h, -1),
                                r_view(c, h, +1))
                for c in chunks:           # op4: v = max(v, r)
                    for h in (0, 1):
                        nc.vector.tensor_max(v_view(c, h), v_view(c, h),
                                             r_view(c, h))
                for c in chunks:           # op5a: w = c*v (w overlays r int.)
                    if solo:
                        nc.scalar.mul(r_view(c, 0), v_view(c, 0), csc(c))
                    elif mode == "act_full":
                        nc.scalar.mul(r_view(c, None), v_view(c, None),
                                      csc(c))
                    elif mode == "act_halves":
                        for h in (0, 1):
                            nc.scalar.mul(r_view(c, h), v_view(c, h), csc(c))
                for c in chunks:           # op5b: u += w
                    if solo:
                        nc.vector.tensor_scalar_mul(r_view(c, 1),
                                                    v_view(c, 1), csc(c))
                        nc.vector.tensor_add(u_view(c, 1), u_view(c, 1),
                                             r_view(c, 1))
                        nc.vector.tensor_add(u_view(c, 0), u_view(c, 0),
                                             r_view(c, 0))
                    else:
                        if mode == "dve_ts":
                            nc.vector.tensor_scalar_mul(
                                r_view(c, None), v_view(c, None), csc(c))
                        for h in (0, 1):
                            nc.vector.tensor_add(u_view(c, h), u_view(c, h),
                                                 r_view(c, h))
                    if not last and OPTS["op1_pos"] == "end":
                        emit_op1(c)

            def emit_tail(c):
                ut = u_t[c % 2]
                vt = v_t[c % 2]
                rows = slice(c * CHUNK, (c + 1) * CHUNK)
                # row pairs first (contiguous operands -> 2x mode)
                u4 = ut[:, UOFF:UOFF + USTR * H].rearrange(
                    "p (h2 two w) -> p h2 two w", h2=H // 2, two=2, w=USTR)
                b1 = vt[:, 0:(H // 2) * W].rearrange(
                    "p (h w) -> p h w", h=H // 2, w=W)
                nc.vector.tensor_add(b1, u4[:, :, 0:1, 0:W],
                                     u4[:, :, 1:2, 0:W])
                # then column pairs (stride-2 operands, 1x), scale + store
                # in halves so ACT scale and the cast-DMA pipeline
                fsc = cs_t[:, 2 * (c % NCHUNK) + 1:2 * (c % NCHUNK) + 2]
                for h in (0, 1):
                    b2 = vt[:, (H // 4) * W * h:
                            (H // 4) * W * (h + 1)].rearrange(
                        "p (h w2 two) -> p h w2 two",
                        h=H // 4, w2=W // 2, two=2)
                    bsum = vt[:, H * 64 + 2048 * h:H * 64 + 2048 * (h + 1)]
                    nc.vector.tensor_add(
                        bsum.rearrange("p (h w) -> p h w", h=32, w=64),
                        b2[:, :, :, 0:1], b2[:, :, :, 1:2])
                    bf = vt[:, H * 64 + 4096 + 2048 * h:
                            H * 64 + 4096 + 2048 * (h + 1)]
                    nc.scalar.mul(bf, bsum, fsc)
                    nc.gpsimd.dma_start(
                        y_d[rows, 2048 * h:2048 * (h + 1)],
                        bf.rearrange("p (a b) -> p a b", a=32, b=64))

            def body():
                load_chunk(0)
                load_chunk(1)
                for k in range(NCHUNK):
                    rows = slice(k * CHUNK, (k + 1) * CHUNK)
                    nc.sync.dma_start(cs_t[:, 2 * k:2 * k + 1], c_d[rows, :])
                    nc.sync.dma_start(cs_t[:, 2 * k + 1:2 * k + 2],
                                      f_d[rows, :])
                if OPTS["op1_pos"] == "end":
                    emit_op1(0)
                    emit_op1(1)
                for t in range(T):
                    emit_steps((0, 1), t, last=(t == T - 1))
                emit_tail(0)
                load_chunk(2)
                emit_tail(1)
                if OPTS["op1_pos"] == "end":
                    emit_op1(2)
                for t in range(T):
                    emit_steps((2,), t, last=(t == T - 1))
                emit_tail(2)

            if R is None:
                body()
            else:
                with tc.For_i(0, R) as _i:
                    body()

    nc.compile()
    return nc


def _get_program():
    if "nc" not in _CACHE:
        _CACHE["nc"] = _build()
    return _CACHE["nc"]


def kernel(x: np.ndarray, pool_strength: np.ndarray) -> np.ndarray:
    from concourse.bass_utils import run_bass_kernel_spmd

    nc = _get_program()

    x16 = np.asarray(x, dtype=np.float16)
    s = np.asarray(pool_strength, dtype=np.float64).reshape(C)
    c_ch = (s / (1.0 - s)).astype(np.float32)                  # [C]
    f_ch = (((1.0 - s) ** T) * 0.25).astype(np.float32)        # [C]
    cvec = np.ascontiguousarray(np.tile(c_ch, B_PER_CORE)[:, None])  # [384,1]
    fvec = np.ascontiguousarray(np.tile(f_ch, B_PER_CORE)[:, None])

    in_maps = []
    for j in range(N_CORES):
        xj = np.ascontiguousarray(
            x16[j * B_PER_CORE:(j + 1) * B_PER_CORE].reshape(IMGS, H * W))
        in_maps.append({"x": xj, "cvec": cvec, "fvec": fvec})

    res = run_bass_kernel_spmd(nc, in_maps, list(range(N_CORES)))

    out = np.empty((B, C, H // 2, W // 2), dtype=np.float32)
    for j in range(N_CORES):
        yj = res.results[j]["y"].reshape(B_PER_CORE, C, H // 2, W // 2)
        out[j * B_PER_CORE:(j + 1) * B_PER_CORE] = yj
    return out
